# revision 6
# baseline (speedup 1.0000x reference)
"""Deformable depthwise conv (DConv) Trainium2 kernel — V3.

V2 + pipelined preamble: the offset conv, field math, wrap transposes,
index build, and gating build are all split by output j-block (half the
image rows), so the jb0 gather stream starts after only half the conv
instead of after the whole serial preamble. While GPSIMD gathers jb0,
PE/DVE/ACT compute the jb1 preamble.

See kernel2 docstring for the V2 engine split (pair-packed gathers on
GPSIMD, DVE bf16 2x gating vs DMA-broadcast gating rows, PE diag-wdg
accumulation with stride-2 rhs).
"""

import os
import numpy as np

import concourse.bass as bass
import concourse.bacc as bacc
import concourse.mybir as mybir
import concourse.tile as tile

f32 = mybir.dt.float32
bf16 = mybir.dt.bfloat16
fp8 = mybir.dt.float8e4
i32 = mybir.dt.int32
i16 = mybir.dt.int16

B, C, H, W = 8, 256, 64, 64
HW = H * W            # 4096
HW2 = HW // 2         # 2048
PAD = 2
PW = W + 2 * PAD      # 68
NPIX = PW * PW        # 4624
KK = 9                # 3x3 taps
NCORES = 8
FBIAS = 7.5           # bias so HW round-to-nearest cast == floor+8

AF = mybir.ActivationFunctionType
ALU = mybir.AluOpType


def _build_nc():
    nc = bacc.Bacc("TRN2", target_bir_lowering=False, debug=False,
                   num_devices=NCORES)
    x_d = nc.dram_tensor("x", [C, H, W], f32, kind="ExternalInput")
    wo_d = nc.dram_tensor("wo", [2, 128, KK, 18], bf16, kind="ExternalInput")
    wdiag_d = nc.dram_tensor("wdiag", [128, 18, 128], bf16,
                             kind="ExternalInput")
    base_d = nc.dram_tensor("base", [18, HW], bf16, kind="ExternalInput")
    ident_d = nc.dram_tensor("ident", [32, 32], bf16, kind="ExternalInput")
    out_d = nc.dram_tensor("out", [C, H, W], f32, kind="ExternalOutput")
    gat_d = nc.dram_tensor("gat_scratch", [2, KK, 2 * HW], bf16,
                           kind="Internal")

    with tile.TileContext(nc) as tc:
        _kernel(tc, out_d, x_d, wo_d, wdiag_d, base_d, ident_d, gat_d)
    nc.compile()
    return nc


def _kernel(tc, out_d, x_d, wo_d, wdiag_d, base_d, ident_d, gat_d):
    nc = tc.nc

    with tc.tile_pool(name="persist", bufs=1) as persist:
        # ---------------- constants ----------------
        wo_sb = [persist.tile([128, KK, 18], bf16, name=f"wo{h}",
                              tag=f"wo{h}") for h in range(2)]
        for h in range(2):
            nc.sync.dma_start(wo_sb[h][:], wo_d[h])
        wdiag = persist.tile([128, 18, 128], bf16, tag="wdiag")
        nc.sync.dma_start(wdiag[:], wdiag_d[:])
        ident = persist.tile([32, 32], bf16, tag="ident")
        nc.sync.dma_start(ident[:], ident_d[:])

        xp2 = [persist.tile([128, 2, NPIX], bf16, name=f"xp2_{h}",
                            tag=f"xp2_{h}") for h in range(2)]
        idxR = persist.tile([128, 2, KK, 2, 128], i16, tag="idxR")
        cpool_cm = tc.tile_pool(name="cpool", bufs=1)
        cpool = cpool_cm.__enter__()
        base = cpool.tile([18, HW], bf16, tag="base")
        nc.sync.dma_start(base[:], base_d[:])
        offs = cpool.tile([18, HW], f32, tag="offs")

        ldpool_cm = tc.tile_pool(name="ldpool", bufs=2)
        ldpool = ldpool_cm.__enter__()
        xstg = []
        for h in range(2):
            # zero the even plane once; interior rows overwritten below
            nc.scalar.memzero(xp2[h][:, 0, :])
            xstg.append(ldpool.tile([128, HW], f32, name=f"xs{h}",
                                    tag="xs"))

        def load_q(q):
            # contiguous DMA chunk into staging (16KB runs, full DMA rate),
            # then strided bf16 convert into the padded even plane.
            for h in range(2):
                nc.sync.dma_start(
                    xstg[h][:, 1024 * q:1024 * (q + 1)],
                    x_d[128 * h:128 * (h + 1),
                        16 * q:16 * (q + 1)].rearrange("c y x -> c (y x)"),
                )
                dst = xp2[h][:, 0, :].rearrange(
                    "p (y x) -> p y x", y=PW, x=PW)[
                    :, PAD + 16 * q:PAD + 16 * (q + 1), PAD:PAD + W]
                nc.scalar.activation(
                    dst,
                    xstg[h][:, 1024 * q:1024 * (q + 1)].rearrange(
                        "p (y x) -> p y x", y=16, x=W),
                    AF.Copy)
                if q == 3:
                    # odd plane = even plane shifted one element
                    nc.scalar.activation(xp2[h][:, 1, 0:NPIX - 1],
                                         xp2[h][:, 0, 1:NPIX], AF.Copy)
                    nc.vector.memset(xp2[h][:, 1, NPIX - 1:NPIX], 0.0)


        fpool_cm = tc.tile_pool(name="fpool", bufs=1)
        fpool = fpool_cm.__enter__()
        fS = fpool.tile([18, HW], bf16, tag="fS")
        omfS = fpool.tile([18, HW], bf16, tag="omfS")
        W18 = fpool.tile([16, 256, 18], bf16, tag="W18")

        psc_cm = tc.tile_pool(name="psc", bufs=4, space=bass.MemorySpace.PSUM)
        psc = psc_cm.__enter__()
        psw_cm = tc.tile_pool(name="psw", bufs=2, space=bass.MemorySpace.PSUM)
        psw = psw_cm.__enter__()
        ftmp_cm = tc.tile_pool(name="ftmp", bufs=2)
        ftmp = ftmp_cm.__enter__()
        ipool_cm = tc.tile_pool(name="ipool", bufs=1)
        ipool = ipool_cm.__enter__()
        gatp_cm = tc.tile_pool(name="gatp", bufs=1)
        gatp = gatp_cm.__enter__()

        xpb3 = [xp2[h][:, 0, :].rearrange("p (y x) -> p y x", y=PW, x=PW)
                for h in range(2)]

        def conv_chunk(n):
            pt = psc.tile([18, 512], f32, tag="convps")
            first = True
            for t in range(KK):
                dy, dx = t // 3, t % 3
                for h in range(2):
                    rhs = xpb3[h][:, (dy + 1) + 8 * n:(dy + 1) + 8 * n + 8,
                                  (dx + 1):(dx + 1) + W]
                    nc.tensor.matmul(pt[:], wo_sb[h][:, t, :], rhs,
                                     start=first,
                                     stop=(t == KK - 1 and h == 1))
                    first = False
            nc.scalar.activation(offs[:, 512 * n:512 * (n + 1)], pt[:],
                                 AF.Copy)

        def fields_chunk(n):
            cs = slice(512 * n, 512 * (n + 1))
            nfi = ftmp.tile([18, 512], i32, tag="nfi")
            nf = ftmp.tile([18, 512], f32, tag="nf")
            bS = ftmp.tile([18, 512], bf16, tag="bS")
            fsub = ftmp.tile([18, 512], f32, tag="fsub")
            nc.vector.tensor_add(offs[:, cs], offs[:, cs], base[:, cs])
            nc.vector.tensor_copy(nfi[:], offs[:, cs])
            nc.vector.tensor_copy(nf[:], nfi[:])
            nc.vector.tensor_tensor(fsub[:], offs[:, cs], nf[:],
                                    ALU.subtract)
            nc.scalar.activation(omfS[:, cs], fsub[:], AF.Copy, bias=0.5,
                                 scale=-1.0)
            nc.scalar.activation(fS[:, cs], fsub[:], AF.Copy, bias=0.5,
                                 scale=1.0)
            nc.scalar.activation(bS[:], nf[:], AF.Copy)
            return bS

        def transp_chunk(n, bS):
            for g4 in range(8):
                pw = psw.tile([16, 4, 18], bf16, tag="wrapps")
                for j in range(4):
                    s = 16 * (4 * g4 + j)
                    nc.tensor.transpose(pw[:, j, :], bS[:, s:s + 16],
                                        ident[0:18, 0:18])
                nc.scalar.activation(
                    W18[:, 32 * n + 4 * g4:32 * n + 4 * g4 + 4, :], pw[:],
                    AF.Copy)

        def idx_chain(jb):
            ss = slice(128 * jb, 128 * (jb + 1))
            ncl = ipool.tile([16, 128, 18], bf16, tag="ncl")
            FF0 = ipool.tile([16, 128, KK], f32, tag="FF0")
            ihf = ipool.tile([16, 128, KK], f32, tag="ihf")
            ihi = ipool.tile([16, 128, KK], i32, tag="ihi")
            nc.vector.tensor_scalar(ncl[:], W18[:, ss, :], 6.0, 72.0,
                                    ALU.max, ALU.min)
            nc.vector.scalar_tensor_tensor(FF0[:], ncl[:, :, 0:KK], 68.0,
                                           ncl[:, :, KK:18], ALU.mult,
                                           ALU.add)
            nc.vector.tensor_scalar(ihf[:], FF0[:], 0.5, -207.25,
                                    ALU.mult, ALU.add)
            nc.vector.tensor_copy(ihi[:], ihf[:])
            nc.vector.tensor_copy(ihf[:], ihi[:])
            # FF0 <- 2312 * (par = FF0 - 2*ih - 414)
            nc.vector.scalar_tensor_tensor(FF0[:], ihf[:], -2.0, FF0[:],
                                           ALU.mult, ALU.add)
            nc.vector.tensor_scalar(FF0[:], FF0[:], -414.0, 2312.0,
                                    ALU.add, ALU.mult)
            # ihf <- idx = ih + 2312*par
            nc.vector.tensor_tensor(ihf[:], ihf[:], FF0[:], ALU.add)
            for yc in range(2):
                dst = idxR[0:16, jb, :, yc, :].rearrange("p k s -> p s k")
                nc.vector.tensor_scalar(dst, ihf[:], 34.0 * yc, 0.0,
                                        ALU.add, ALU.add)
            for st in (16, 32, 64):
                nc.sync.dma_start(
                    idxR[st:2 * st, jb].rearrange("p a b c -> p (a b c)"),
                    idxR[0:st, jb].rearrange("p a b c -> p (a b c)"))

        def gat_build(jb, eng=None):
            cs = slice(HW2 * jb, HW2 * (jb + 1))
            xx = gatp.tile([KK, 2 * HW2], bf16, tag="xx")
            nc.sync.dma_start(xx[:, 0:HW2], omfS[KK:18, cs])
            nc.sync.dma_start(xx[:, HW2:2 * HW2], fS[KK:18, cs])
            p1 = gatp.tile([KK, 2 * HW2], bf16, tag="p1")
            p2 = gatp.tile([KK, 2 * HW2], bf16, tag="p2")
            p1v = p1[:].rearrange("p (j two) -> p two j", two=2)
            p2v = p2[:].rearrange("p (j two) -> p two j", two=2)
            eng2 = nc.vector if jb == 1 else nc.gpsimd
            for dx in range(2):
                wx = xx[:, HW2 * dx:HW2 * (dx + 1)]
                nc.gpsimd.tensor_tensor(p1v[:, dx, :], omfS[0:KK, cs], wx,
                                        ALU.mult)
                eng2.tensor_tensor(p2v[:, dx, :], fS[0:KK, cs], wx,
                                   ALU.mult)
            nc.sync.dma_start(gat_d[0, :, HW * jb:HW * (jb + 1)], p1[:])
            nc.sync.dma_start(gat_d[1, :, HW * jb:HW * (jb + 1)], p2[:])

        # ---- pipelined preamble: jb0 ----
        # conv chunk n reads orig rows [8n-1, 8n+9] -> needs q <= (8n+9)//16
        load_q(0)
        conv_chunk(0)
        load_q(1)
        conv_chunk(1)
        fields_chunk_out = [fields_chunk(0)]
        conv_chunk(2)
        fields_chunk_out.append(fields_chunk(1))
        transp_chunk(0, fields_chunk_out[0])
        load_q(2)
        conv_chunk(3)
        fields_chunk_out.append(fields_chunk(2))
        transp_chunk(1, fields_chunk_out[1])
        fields_chunk_out.append(fields_chunk(3))
        transp_chunk(2, fields_chunk_out[2])
        transp_chunk(3, fields_chunk_out[3])
        idx_chain(0)
        gat_build(0)

        # ---- jb1 preamble (overlaps jb0 main loop at runtime) ----
        load_q(3)
        pend = []
        for n in range(4, 8):
            conv_chunk(n)
            pend.append((n, fields_chunk(n)))
            if n >= 5:
                transp_chunk(*pend.pop(0))
        transp_chunk(*pend.pop(0))
        idx_chain(1)
        gat_build(1)

        gatp_cm.__exit__(None, None, None)
        ipool_cm.__exit__(None, None, None)
        ftmp_cm.__exit__(None, None, None)
        psw_cm.__exit__(None, None, None)
        psc_cm.__exit__(None, None, None)
        fpool_cm.__exit__(None, None, None)
        ldpool_cm.__exit__(None, None, None)
        cpool_cm.__exit__(None, None, None)

        # ---------------- main loops ----------------
        srcs = [xp2[h][:].rearrange("p t f -> p (t f)").bitcast(
            f32).unsqueeze(2) for h in range(2)]
        with (
            tc.tile_pool(name="pso", bufs=1, space=bass.MemorySpace.PSUM) as pso,
            tc.tile_pool(name="rpool", bufs=5) as rpool,
            tc.tile_pool(name="gpool", bufs=3) as gpool,
            tc.tile_pool(name="opool", bufs=1) as opool,
        ):
            for jb in range(2):
                ops = [pso.tile([128, HW2], f32, name=f"outps{h}",
                                tag=f"outps{h}") for h in range(2)]
                reps = {}
                for k in range(KK):
                    for yc in range(2):
                        rt = rpool.tile([128, HW], bf16, tag="rep")
                        nc.sync.dma_start(
                            rt[:],
                            gat_d[yc, k, HW * jb:HW * (jb + 1)].unsqueeze(
                                0).broadcast_to([128, HW]))
                        reps[(k, yc)] = rt
                idxflat = idxR[:, jb].rearrange("p k y s -> p (k y s)")
                for c in range(5):
                    ncall = 8192 if c < 4 else 4096
                    nq = 4 if c < 4 else 2
                    for h in range(2):
                        gt = gpool.tile([128, 2 * HW, 1], f32, tag="G")
                        nc.gpsimd.ap_gather(
                            gt[:, 0:ncall], srcs[h],
                            idxflat[:, 512 * c:512 * c + ncall // 16],
                            channels=128, num_elems=NPIX, d=1,
                            num_idxs=ncall)
                        gb = gt[:].rearrange(
                            "p f one -> p (f one)").bitcast(bf16)
                        for q in range(nq):
                            k = 2 * c + q // 2
                            yc = q % 2
                            hv = gb[:, HW * q:HW * (q + 1)]
                            nc.vector.tensor_tensor(
                                hv, hv, reps[(k, yc)][:], ALU.mult)
                            hj = hv.rearrange("p (j two) -> p two j", two=2)
                            for dx in range(2):
                                for m in range(4):
                                    ms = slice(512 * m, 512 * (m + 1))
                                    nc.tensor.matmul(
                                        ops[h][:, ms],
                                        wdiag[:, 2 * k + h, :],
                                        hj[:, dx, ms],
                                        start=(c == 0 and q == 0
                                               and dx == 0),
                                        stop=(c == 4 and q == 1
                                              and dx == 1),
                                    )
                for h in range(2):
                    osb = opool.tile([128, HW2], f32, tag="osb")
                    nc.scalar.activation(osb[:], ops[h][:], AF.Copy)
                    nc.sync.dma_start(
                        out_d[128 * h:128 * (h + 1),
                              32 * jb:32 * (jb + 1)].rearrange(
                            "c y x -> c (y x)"),
                        osb[:])


def _host_inputs(w_offset, w_deform):
    """Build per-core constant inputs (everything except the image)."""
    import ml_dtypes
    wo = np.empty((2, 128, KK, 18), np.float32)
    for h in range(2):
        for t in range(KK):
            ky, kx = t // 3, t % 3
            for m in range(18):
                oc = 2 * m if m < 9 else 2 * (m - 9) + 1
                wo[h, :, t, m] = w_offset[oc, 128 * h:128 * (h + 1), ky, kx]
    wd = w_deform.reshape(C, KK)
    wdiag = np.zeros((128, 18, 128), np.float32)
    for k in range(KK):
        for h in range(2):
            np.fill_diagonal(wdiag[:, 2 * k + h, :],
                             wd[128 * h:128 * (h + 1), k])
    base = np.empty((18, HW), np.float32)
    yy, xx = np.mgrid[0:H, 0:W]
    for k in range(KK):
        ky, kx = k // 3, k % 3
        base[k, :] = (yy + ky - 1).reshape(-1) + FBIAS
        base[9 + k, :] = (xx + kx - 1).reshape(-1) + FBIAS
    ident = np.eye(32, dtype=ml_dtypes.bfloat16)
    return {"wo": wo.astype(ml_dtypes.bfloat16),
            "wdiag": wdiag.astype(ml_dtypes.bfloat16),
            "base": base.astype(ml_dtypes.bfloat16), "ident": ident}


_NC_CACHE = None
LAST_EXEC_NS = None


def kernel(x, w_offset, w_deform):
    global _NC_CACHE
    x = np.asarray(x, np.float32)
    w_offset = np.asarray(w_offset, np.float32)
    w_deform = np.asarray(w_deform, np.float32)

    consts = _host_inputs(w_offset, w_deform)
    in_maps = [dict(consts, x=np.ascontiguousarray(x[i])) for i in range(B)]

    if _NC_CACHE is None:
        _NC_CACHE = _build_nc()
    nc = _NC_CACHE

    from concourse.bass_utils import run_bass_kernel_spmd
    global LAST_EXEC_NS
    trace = bool(os.environ.get("BASS_TRACE"))
    res = run_bass_kernel_spmd(nc, in_maps, core_ids=list(range(NCORES)),
                               trace=trace)
    LAST_EXEC_NS = res.exec_time_ns
    return np.stack([res.results[i]["out"] for i in range(B)], axis=0)


if __name__ == "__main__":
    import jax
    import reference
    cpu = jax.devices("cpu")[0]
    with jax.default_device(cpu):
        jinputs = reference.setup_inputs()
        jexpected = reference.reference(**jinputs)
    inputs = {k: np.asarray(jax.device_get(v)) for k, v in jinputs.items()}
    expected = np.asarray(jax.device_get(jexpected))
    actual = kernel(**inputs)
    rel = np.linalg.norm(actual - expected) / np.linalg.norm(expected)
    print("Relative error:", rel)
    print("max abs diff:", np.abs(actual - expected).max())
    from concourse.timeline_sim import TimelineSim
    print("HW exec time:", round(TimelineSim(_NC_CACHE).simulate()), "ns")
